# revision 24
# baseline (speedup 1.0000x reference)
"""Trainium2 Bass kernel for BatchedModelManifoldGeodesicFlow.

Closed-form math (per sample), derived from the reference's autodiff:
  f(x) = tanh(x@W1 + b1)@W2 + b2,  J = jacrev(f)(x) = W2^T diag(d) W1^T
  with h = tanh(x@W1+b1), d = 1-h^2, e = -2*h*d, K = W1^T W1, L = W2 W2^T.
  V := L diag(d) K,  W := K diag(d) L (= V^T),  U := K diag(d) V,
  F := (K.*L).*U + K.*V.*W
    ||dG||^2 = 8 * (h.*d)^T F (h.*d)
  Christoffel contraction -> small matvecs:
    S1 = W2^T [ e.*cv.*w + d.*(K (e.*g.*w)) ],  0.5*S2 = W1 (e.*g.*cv)
    w = W1^T v, g = W2 v, cv = K (d.*g)
    a = (0.5*S2 - S1) / ((||dG||+1e-6) * (||v||+1e-6))
  out = concat([v, a - 0.1*dev], axis=0)

The 1/sqrt(8) of the norm is folded into e (S1, 0.5*S2 are homogeneous
degree-1 in e), and the 0.5 of S2 into the host-shipped 0.5*W1^T.

Layouts: host ships pre-transposed bf16 weight views; K/L are computed
on-device; the heavy H x H x H products (V, W, U) run as fp8e4
DoubleRow matmuls (two k-tiles per instruction, 2x rate); per-sample
work is batched into wide moving operands.

Sharding: pure data parallel, batch 16 -> 2 samples per core on 8 cores.
"""

import sys

if "/opt/trn_rl_repo" not in sys.path:
    sys.path.insert(0, "/opt/trn_rl_repo")

import numpy as np

import concourse.bacc as bacc
import concourse.tile as tile
from concourse import mybir
from concourse.tile import add_dep_helper
from concourse.masks import make_identity

N = 128
H = 256
B = 16
NCORES = 8
BLOC = B // NCORES  # 2 samples per core

F32 = mybir.dt.float32
BF16 = mybir.dt.bfloat16
F8 = mybir.dt.float8e4
AF = mybir.ActivationFunctionType
OP = mybir.AluOpType
AX = mybir.AxisListType
DR = mybir.MatmulPerfMode.DoubleRow

ISQRT8 = 0.35355339059327373  # 1/sqrt(8), folded into e


def ts(i, sz=128):
    return slice(i * sz, (i + 1) * sz)


def build_nc():
    nc = bacc.Bacc(trn_type="TRN2", enable_partition_id=False)

    # column-major vectors + misc: cols dev(0:2) x0(2:4) x1(4:6) vel(6:8)
    # | t(8) | b1c(9:11) | pad(11)
    d_vm = nc.dram_tensor("vm", [128, 12], F32, kind="ExternalInput")
    # bf16 weight views: [w1b | w2t], each 256 cols
    d_wb = nc.dram_tensor("wb", [128, 2, H], BF16, kind="ExternalInput")
    d_out = nc.dram_tensor("out_bot", [BLOC, N + 4], F32, kind="ExternalOutput")

    with tile.TileContext(nc) as tc:
        with (
            tc.tile_pool(name="consts", bufs=1) as consts,
            tc.tile_pool(name="work", bufs=1) as work,
            tc.tile_pool(name="pbig", bufs=4, space="PSUM") as pbig,
            tc.tile_pool(name="psmall", bufs=2, space="PSUM") as psmall,
            tc.tile_pool(name="prow", bufs=1, space="PSUM") as prow,
        ):
            _emit(nc, consts, work, pbig, psmall, prow,
                  d_vm, d_wb, d_out)

    nc.compile()
    return nc


def _emit(nc, consts, work, pbig, psmall, prow,
          d_vm, d_wb, d_out):
    # ---------------- input DMA --------------------------------------
    sb_vm = consts.tile([128, 12], F32)
    sb_wb = consts.tile([128, 2, H], BF16)
    nc.sync.dma_start(out=sb_vm, in_=d_vm[:, :])
    nc.scalar.dma_start(out=sb_wb[:, 0, :], in_=d_wb[:, 0, :])
    nc.gpsimd.dma_start(out=sb_wb[:, 1, :], in_=d_wb[:, 1, :])

    # engine warm-up before any input lands: junk matmuls/vector ops keep
    # the PE/DVE clocks ramped so the first real ops run at speed
    junk = consts.tile([128, 4, H], BF16)
    nc.vector.memset(junk[:, 0:2, :], 1.0)
    pwarm = pbig.tile([128, 2, H], F32, tag="big")
    for r in range(5):
        nc.tensor.matmul(
            pwarm, junk[:, 0, 0:128], junk[:, 0:2, :], start=True, stop=True,
            skip_group_check=True,
        )
    for r in range(3):
        nc.vector.tensor_mul(junk[:, 2:4, :], junk[:, 0:2, :], junk[:, 0:2, :])

    # identity for the on-chip weight transposes
    ident = consts.tile([128, 128], BF16)
    make_identity(nc, ident)
    t128 = sb_vm[:, 8:9]
    sb_b1c = sb_vm[:, 9:11]
    w4c = sb_vm[:, 11:12]            # 4*t*(1-t), host-computed scalar
    w1b = sb_wb[:, 0, :]
    w2t = sb_wb[:, 1, :]
    devc, x0c = sb_vm[:, 0:2], sb_vm[:, 2:4]
    x1c, velc = sb_vm[:, 4:6], sb_vm[:, 6:8]

    ones1 = consts.tile([128, 1], F32)
    nc.vector.memset(ones1, 1.0)

    # ---------------- x along the path (vector, tiny) ------------------
    mv_b = work.tile([128, 4], BF16)  # cols: x(2) | vel(2)
    dxc = work.tile([128, BLOC], F32)
    nc.vector.tensor_sub(dxc, x1c, x0c)
    nc.vector.tensor_copy(mv_b[:, 2:4], velc)
    xcf = work.tile([128, BLOC], F32)
    nc.vector.scalar_tensor_tensor(
        out=xcf, in0=dxc, scalar=t128, in1=x0c, op0=OP.mult, op1=OP.add
    )
    nc.vector.scalar_tensor_tensor(
        out=mv_b[:, 0:2], in0=devc, scalar=w4c, in1=xcf, op0=OP.mult, op1=OP.add
    )
    velc_b = mv_b[:, 2:4]

    # on-chip weight views: w2tile = W2 row-tiles, w1th = -W1^T tiles
    # (PE transposes in the pre-arrival idle window)
    wder = consts.tile([128, 2, 2, 128], BF16)  # [{w2tile, w1th}, t, j]
    w2tile = wder[:, 0, :, :].reshape([128, 256]) if False else None
    p_t1 = psmall.tile([128, 2, 128], BF16, tag="small")
    for t in range(2):
        nc.tensor.transpose(out=p_t1[:, t, :], in_=w2t[:, ts(t)], identity=ident)
    nc.vector.tensor_copy(wder[:, 0, :, :], p_t1)
    p_t2 = psmall.tile([128, 2, 128], BF16, tag="small")
    for t in range(2):
        nc.tensor.transpose(out=p_t2[:, t, :], in_=w1b[:, ts(t)], identity=ident)
    nc.vector.tensor_scalar_mul(wder[:, 1, :, :], p_t2, -1.0)
    w2tile = wder[:, 0, :, :]
    w1th = wder[:, 1, :, :]

    # ---------------- u|w matmuls, tanh; K, L --------------------------
    # p_uw[:, t, 0:2] = u cols, [:, t, 2:4] = w = W1^T v cols
    p_uw = psmall.tile([128, 2, 4], F32, tag="small")
    for t in range(2):
        nc.tensor.matmul(p_uw[:, t, :], w1b[:, ts(t)], mv_b, start=True, stop=True)
    p_k = pbig.tile([128, 2, H], F32, tag="big")   # K rows-blocks
    for t in range(2):
        nc.tensor.matmul(p_k[:, t, :], w1b[:, ts(t)], w1b, start=True, stop=True)
    p_l = pbig.tile([128, 2, H], F32, tag="big")   # L rows-blocks
    for t in range(2):
        nc.tensor.matmul(p_l[:, t, :], w2t[:, ts(t)], w2t, start=True, stop=True)
    # g = W2 v  (cols per t)
    p_g = psmall.tile([128, 2, BLOC], F32, tag="small")
    for t in range(2):
        nc.tensor.matmul(p_g[:, t, :], w2t[:, ts(t)], velc_b, start=True, stop=True)

    h_c = work.tile([128, 2, BLOC], F32)
    i_tanh = []
    for t in range(2):
        i_tanh.append(nc.scalar.activation(
            out=h_c[:, t, :], in_=p_uw[:, t, 0:2], func=AF.Tanh,
            bias=sb_b1c[:, t : t + 1], scale=1.0,
        ))
    # lv layout [128, t, {v0, l, v1}, H]: W|U moving slice s:s+2 gives
    # s=0 -> [V_0 | L] and s=1 -> [L | V_1].  lv-l on scalar after tanh.
    lv = consts.tile([128, 2, 3, H], F8)
    i_lvl = []
    for t in range(2):
        i_lvl.append(nc.scalar.copy(out=lv[:, t, 1, :], in_=p_l[:, t, :]))
    add_dep_helper(i_lvl[0].ins, i_tanh[1].ins, sync=False,
                   reason="keep both tanhs ahead of lv-l on scalar")

    # ---------------- vector: kb, d, kf8, ehd --------------------------
    # kb: the single PSUM read of K; everything else derives from SBUF
    kb = consts.tile([128, 2, H], BF16)
    nc.vector.tensor_copy(kb, p_k)
    d_c = work.tile([128, 2, BLOC], F32)
    nc.vector.tensor_mul(d_c, h_c, h_c)
    nc.vector.tensor_scalar(
        out=d_c, in0=d_c, scalar1=-1.0, scalar2=1.0, op0=OP.mult, op1=OP.add
    )
    # kf8x[:, t, :] = d_s .* K rows (fp8)
    kf8a = consts.tile([128, 2, H], F8)
    kf8b = consts.tile([128, 2, H], F8)
    for t in range(2):
        nc.vector.tensor_scalar_mul(kf8a[:, t, :], kb[:, t, :], d_c[:, t, 0:1])
    for t in range(2):
        nc.vector.tensor_scalar_mul(kf8b[:, t, :], kb[:, t, :], d_c[:, t, 1:2])
    kf8s = [kf8a, kf8b]
    ehd_c = work.tile([128, 2, BLOC, 2], F32)  # [...,0]=e', [...,1]=h*d
    nc.vector.tensor_mul(ehd_c[:, :, :, 1], h_c, d_c)
    # e' = -2*h*d/sqrt(8): folds the norm's 8x into the contraction path
    nc.vector.tensor_scalar_mul(ehd_c[:, :, :, 0], ehd_c[:, :, :, 1], -2.0 * ISQRT8)
    e_c = ehd_c[:, :, :, 0]
    hd_c = ehd_c[:, :, :, 1]
    ehd_b = work.tile([128, 2, BLOC, 2], BF16)
    nc.vector.tensor_copy(ehd_b, ehd_c)

    # ---------------- norm path: V, W|U (fp8 DoubleRow) ----------------
    # V psum per sample (own bank: the s0 copies don't wait on V_s1)
    p_vss = []
    for s in range(BLOC):
        p_v = pbig.tile([128, 2, H], F32, tag="big")
        p_vss.append(p_v)
        for pt in range(2):
            nc.tensor.matmul(
                p_v[:, pt, :], lv[:, :, 1, ts(pt)], kf8s[s],
                start=True, stop=True, perf_mode=DR,
            )
    # V psum -> fp8 lv slots: s0 on vector (one strided op), s1 on scalar
    nc.vector.tensor_copy(lv[:, :, 0, :], p_vss[0])
    i_lvv1 = nc.scalar.copy(out=lv[:, :, 2, :], in_=p_vss[1])
    # multiplier slots [kv1, klb, kv0] (bf16)
    mslot = consts.tile([128, 2, 3, H], BF16)
    for pt in range(2):
        nc.gpsimd.tensor_mul(mslot[:, pt, 1, :], lv[:, pt, 1, :], kb[:, pt, :])
    nc.vector.tensor_mul(mslot[:, :, 2, :], lv[:, :, 0, :], kb)
    for pt in range(2):
        nc.gpsimd.tensor_mul(mslot[:, pt, 0, :], lv[:, pt, 2, :], kb[:, pt, :])

    # S-path elementwise that only needs d/e/g/w (fills the V-wait gap)
    dgy = work.tile([128, 2, 2, BLOC], BF16)  # [t, {dg, y}, s]
    eg = work.tile([128, 2, BLOC], F32)
    ew = work.tile([128, 2, BLOC], F32)
    nc.vector.tensor_mul(dgy[:, :, 0, :], p_g, d_c)
    nc.vector.tensor_mul(eg, p_g, e_c)
    nc.vector.tensor_mul(ew, e_c, p_uw[:, :, 2:4])
    nc.vector.tensor_mul(dgy[:, :, 1, :], eg, p_uw[:, :, 2:4])

    # cvky (PE fills the WU-wait gap)
    p_cvky = psmall.tile([128, 2, 2, BLOC], F32, tag="small")  # [mt, {cv,ky}, s]
    for mt in range(2):
        for qt in range(2):
            nc.tensor.matmul(
                p_cvky[:, mt, :, :], kb[:, qt, ts(mt)], dgy[:, qt, :, :],
                start=(qt == 0), stop=(qt == 1),
            )
    p_cv = p_cvky[:, :, 0, :]
    p_ky = p_cvky[:, :, 1, :]

    # [W_s | U_s] rows-pt, fused (q|r) multiply, direct matvec accumulation
    p_f = psmall.tile([128, BLOC, 2], F32, tag="small")  # [s, mt]
    qrows = []
    for s in range(BLOC):
        qrow = []
        for pt in range(2):
            p_wu = pbig.tile([128, 2, H], F32, tag="big")
            nc.tensor.matmul(
                p_wu, kf8s[s][:, :, ts(pt)], lv[:, :, s : s + 2, :],
                start=True, stop=True, perf_mode=DR,
            )
            qr = work.tile([128, 2, H], BF16, tag=f"qr{s}{pt}")
            nc.vector.tensor_mul(qr, p_wu, mslot[:, pt, 1 - s : 3 - s, :])
            qrow.append(qr)
        qrows.append(qrow)
        for mt in range(2):
            idx = 0
            for pt in range(2):
                for h in range(2):
                    nc.tensor.matmul(
                        p_f[:, s, mt : mt + 1], qrow[pt][:, h, ts(mt)],
                        ehd_b[:, pt, s, 1:2],
                        start=(idx == 0), stop=(idx == 3),
                    )
                    idx += 1
        if s == 0:
            # S-path smalls slot between the two qr pairs on vector
            z2 = work.tile([128, 2, BLOC], BF16)
            nc.vector.tensor_mul(z2, eg, p_cv)            # e.*g.*cv
            i1 = work.tile([128, 2, BLOC], F32)
            nc.vector.tensor_mul(i1, ew, p_cv)            # e.*w.*cv
            i2 = work.tile([128, 2, BLOC], F32)
            nc.vector.tensor_mul(i2, d_c, p_ky)           # d.*(K y)
            inner = work.tile([128, 2, BLOC], BF16)
            nc.vector.tensor_add(inner, i1, i2)

    # one accumulation group: p_out[:, 0:128] = S1 - 0.5*S2 (w1t is
    # host-negated), i.e. -comb; the host flips the sign
    p_out = prow.tile([BLOC, N + 4], F32, tag="rows")
    nc.tensor.matmul(p_out[:, 0:N], inner[:, 0, :], w2tile[:, 0, :],
                     start=True, stop=False)
    nc.tensor.matmul(p_out[:, 0:N], inner[:, 1, :], w2tile[:, 1, :],
                     start=False, stop=False)
    nc.tensor.matmul(p_out[:, 0:N], z2[:, 0, :], w1th[:, 0, :],
                     start=False, stop=False)
    nc.tensor.matmul(p_out[:, 0:N], z2[:, 1, :], w1th[:, 1, :],
                     start=False, stop=True)

    # ---------------- norm scalars & output ----------------------------
    # scr[p, s, mt] = p_f[p, s, mt] * hd[p, mt, s]; reduce innermost (mt)
    scr = work.tile([128, BLOC, 2], F32)
    nc.vector.tensor_mul(scr, p_f, ehd_c[:, :, :, 1].transpose([0, 2, 1]))
    acc = work.tile([128, BLOC], F32)
    with nc.allow_low_precision("f32 accum of 2-col reduce, fine"):
        nc.vector.reduce_sum(acc, scr, axis=AX.X)
    # hd^T F hd column sums -> n2 (same bank, its own group); host does
    # the final per-sample scalar normalization during the gather
    nc.tensor.matmul(p_out[:, N : N + 1], acc, ones1, start=True, stop=True)
    outp = work.tile([BLOC, N + 4], F32)
    nc.vector.tensor_copy(outp, p_out)
    nc.sync.dma_start(out=d_out[:, :], in_=outp)


_NC_CACHE = None


def _get_nc():
    global _NC_CACHE
    if _NC_CACHE is None:
        _NC_CACHE = build_nc()
    return _NC_CACHE


def make_in_maps(inputs):
    """Shard full inputs into per-core input maps (layout-only host prep)."""
    import ml_dtypes

    state = np.asarray(inputs["state_batch"], dtype=np.float32)
    x0 = np.asarray(inputs["x0_batch"], dtype=np.float32)
    x1 = np.asarray(inputs["x1_batch"], dtype=np.float32)
    W1 = np.asarray(inputs["W1"], dtype=np.float32)
    W2 = np.asarray(inputs["W2"], dtype=np.float32)
    b1 = np.asarray(inputs["b1"], dtype=np.float32)
    t = np.float32(np.asarray(inputs["t"]).reshape(()))
    dev, vel = state[:B], state[B:]



    bf = ml_dtypes.bfloat16
    wb = np.empty((128, 2, H), dtype=bf)
    wb[:, 0, :] = W1.astype(bf)                                   # w1b
    wb[:, 1, :] = np.ascontiguousarray(W2.T).astype(bf)           # w2t
    wb = np.ascontiguousarray(wb)

    in_maps = []
    for c in range(NCORES):
        sl = slice(c * BLOC, (c + 1) * BLOC)
        vm = np.empty((128, 12), np.float32)
        vm[:, 0:8] = np.concatenate([dev[sl], x0[sl], x1[sl], vel[sl]], axis=0).T
        vm[:, 8] = t
        vm[:, 9:11] = b1.reshape(2, 128).T
        vm[:, 11] = 4.0 * t * (1.0 - t)
        in_maps.append(
            {
                "vm": np.ascontiguousarray(vm),
                "wb": wb,
            }
        )
    return in_maps, vel, dev


def kernel(**inputs) -> np.ndarray:
    from concourse.bass_utils import run_bass_kernel_spmd

    nc = _get_nc()
    in_maps, vel, dev = make_in_maps(inputs)
    res = run_bass_kernel_spmd(nc, in_maps, core_ids=list(range(NCORES)))
    outp = np.concatenate([res.results[c]["out_bot"] for c in range(NCORES)], axis=0)
    comb, n2 = -outp[:, :N], outp[:, N]
    # a = sqrt(8)*comb / ((sqrt(8*n2)+eps)*(||v||+eps)); rest = -0.1*dev
    vn = np.linalg.norm(vel.astype(np.float64), axis=1)
    denom = (np.sqrt(np.maximum(n2, 0.0)) + 1e-6 / np.sqrt(8.0)) * (vn + 1e-6)
    bottom = comb / denom[:, None] - 0.1 * dev
    return np.concatenate([vel, bottom], axis=0).astype(np.float32)
